# revision 17
# baseline (speedup 1.0000x reference)
"""ChannelBlock (dense transformer block with channel/cross-covariance attention)
Trainium2 Bass kernel, data-parallel over batch across 8 NeuronCores.

Contract: kernel(**inputs) takes FULL unsharded inputs (np arrays), returns the
FULL output [8, 4096, 256] float32.
"""

import os

import numpy as np

import concourse.bass as bass
import concourse.bass_utils as _bu
import concourse.tile as tile
from concourse import masks, mybir
from concourse.bass_utils import run_bass_kernel_spmd
from concourse.vector_clock import ScopedClock
import bass_rust

# Optionally re-enable walrus' LDWEIGHTS optimization (off by default in this
# container's compile driver); gated so it can be A/B tested.
if os.environ.get("BASS_LDW_OPT", "0") == "1" and not getattr(
    _bu, "_ldw_patched", False
):
    _orig_run_command = _bu.run_command

    def _run_command_ldw(cmd, **kw):
        if isinstance(cmd, list):
            cmd = [
                "--enable-ldw-opt=true" if c == "--enable-ldw-opt=false" else c
                for c in cmd
            ]
        return _orig_run_command(cmd, **kw)

    _bu.run_command = _run_command_ldw
    _bu._ldw_patched = True

# ----------------------------------------------------------------------------
# Workaround: this container's walrus (CoreV3) only supports ONE sync-wait
# command on TPB_CTRL instructions (Drain).  Tile's kernel-tail drain piles all
# outstanding proc waits onto a single Drain -> split into a chain of Drains
# with one wait each.
# ----------------------------------------------------------------------------
_MAX_DRAIN_WAITS = 1


def _patched_drain_and_barrier(self, tick_clock, wait_clock):
    drain_inst = self.nc.sync.drain()
    wait_clock.add_sem_waits(
        drain_inst.ins, ScopedClock({None: tick_clock.global_clock})
    )
    mi = drain_inst.ins
    si = mi.sync_info
    waits = list(si.on_wait) if si else []
    if len(waits) > _MAX_DRAIN_WAITS:
        mi.sync_info = bass_rust.SyncInfo(
            on_wait=waits[:_MAX_DRAIN_WAITS], on_update=list(si.on_update)
        )
        for i in range(_MAX_DRAIN_WAITS, len(waits), _MAX_DRAIN_WAITS):
            extra = self.nc.sync.drain()
            extra.ins.sync_info = bass_rust.SyncInfo(
                on_wait=waits[i : i + _MAX_DRAIN_WAITS], on_update=[]
            )
    self.nc.all_engine_barrier()
    popped = self.nc._tile_sem_poison_stack.pop()
    assert popped is self._sem_poison
    self.nc.clear_and_free_semaphores(list(self.sems.allocated().values()))
    self.nc.all_engine_barrier()


tile.TileContext._drain_and_barrier = _patched_drain_and_barrier

_nop_counter = [0]


def _split_sync_waits(nc, cap=1):
    """Walrus in this container rejects instructions with more than `cap`
    sync-wait commands.  Hoist excess waits onto same-engine NOPs inserted
    immediately before the instruction (engine streams are in-order, so the
    semantics are unchanged)."""
    for f in nc.m.functions:
        for blk in f.blocks:
            changed = False
            new = []
            for inst in blk.instructions:
                si = inst.sync_info
                waits = list(si.on_wait) if si is not None else []
                # ldw-opt rejects Ldweights carrying sync waits; hoist them.
                is_ldw = inst.__class__.__name__ == "InstLdweights"
                eff_cap = 0 if (is_ldw and waits) else cap
                if len(waits) > eff_cap:
                    if is_ldw:
                        excess, keep = waits, []
                    else:
                        excess, keep = waits[:-cap], waits[-cap:]
                    for j in range(0, len(excess), cap):
                        _nop_counter[0] += 1
                        nop = mybir.InstNoOp(
                            name=f"NW-{_nop_counter[0]}", ins=[], outs=[]
                        )
                        nop.engine = inst.engine
                        nop.sync_info = bass_rust.SyncInfo(
                            on_wait=excess[j : j + cap], on_update=[]
                        )
                        new.append(nop)
                    inst.sync_info = bass_rust.SyncInfo(
                        on_wait=keep, on_update=list(si.on_update)
                    )
                    changed = True
                new.append(inst)
            if changed:
                blk.instructions = new

# ----------------------------------------------------------------------------
# Problem constants (hardcoded per the task contract)
# ----------------------------------------------------------------------------
B = 8
N = 4096
C = 256
H = 8
HD = C // H  # 32
HID = 1024
EPS = 1e-5
P = 128
NTILES = N // P  # 32
NCHUNKS = N // 512  # 8

F32 = mybir.dt.float32
BF16 = mybir.dt.bfloat16
FP8 = mybir.dt.float8e4
NP_BF16 = mybir.dt.np(BF16)
NP_FP8 = mybir.dt.np(FP8)
USE_FP8 = os.environ.get("BASS_FP8", "0") == "1"
FP8_SCALE = 64.0

AF = mybir.ActivationFunctionType
ALU = mybir.AluOpType
AX = mybir.AxisListType


def _build_nc(has_bkv, has_bproj, has_bfc2):
    """Build the per-core Bass program (SPMD: all cores run the same NEFF)."""
    nc = bass.Bass()

    # ---- DRAM I/O ----
    x_d = nc.declare_dram_parameter("x", [N, C], F32, isOutput=False)
    wqkv_d = nc.declare_dram_parameter("wqkv", [2, P, 3 * C], BF16, isOutput=False)
    wproj_d = nc.declare_dram_parameter("wproj", [2, P, C], BF16, isOutput=False)
    mlp_dt = FP8 if USE_FP8 else BF16
    w1_d = nc.declare_dram_parameter("w1", [2, P, HID], mlp_dt, isOutput=False)
    w2_d = nc.declare_dram_parameter("w2", [8, P, C], mlp_dt, isOutput=False)
    bq_d = nc.declare_dram_parameter("bq", [2, P, 1], F32, isOutput=False)
    b1_d = nc.declare_dram_parameter("b1", [8, P, 1], F32, isOutput=False)
    bkv_d = nc.declare_dram_parameter("bkv", [1, 2 * C], BF16, isOutput=False)
    bproj_d = nc.declare_dram_parameter("bproj", [1, C], BF16, isOutput=False)
    bfc2_d = nc.declare_dram_parameter("bfc2", [1, C], BF16, isOutput=False)
    out_d = nc.declare_dram_parameter("out", [N, C], F32, isOutput=True)

    with tile.TileContext(nc) as tc:
        import contextlib

        ctx = contextlib.ExitStack()
        with ctx:
            const = ctx.enter_context(tc.tile_pool(name="const", bufs=1))
            xres = ctx.enter_context(tc.tile_pool(name="xres", bufs=1))
            stats = ctx.enter_context(tc.tile_pool(name="stats", bufs=4))
            work = ctx.enter_context(tc.tile_pool(name="work", bufs=3))
            kvp = ctx.enter_context(tc.tile_pool(name="kvp", bufs=3))
            big = ctx.enter_context(tc.tile_pool(name="bigbuf", bufs=2))
            outp = ctx.enter_context(tc.tile_pool(name="outp", bufs=3))
            ps_big = ctx.enter_context(
                tc.tile_pool(name="ps_big", bufs=2, space="PSUM")
            )
            ps_small = ctx.enter_context(
                tc.tile_pool(name="ps_small", bufs=2, space="PSUM")
            )
            ps_tp = ctx.enter_context(tc.tile_pool(name="ps_tp", bufs=2, space="PSUM"))
            ps_attn = ctx.enter_context(
                tc.tile_pool(name="ps_attn", bufs=1, space="PSUM")
            )

            # ---- constants / weights in SBUF ----
            ident = const.tile([P, P], BF16)
            masks.make_identity(nc, ident[:])
            ones_row = const.tile([1, P], BF16)
            nc.vector.memset(ones_row[:], 1.0)
            eps_t = const.tile([P, 1], F32)
            nc.vector.memset(eps_t[:], EPS)

            wqkv = const.tile([P, 2, 3 * C], BF16)
            wproj = const.tile([P, 2, C], BF16)
            w1 = const.tile([P, 2, HID], mlp_dt)
            w2 = const.tile([P, 8, C], mlp_dt)
            for c in range(2):
                nc.sync.dma_start(out=wqkv[:, c, :], in_=wqkv_d[c])
                nc.sync.dma_start(out=wproj[:, c, :], in_=wproj_d[c])
                nc.sync.dma_start(out=w1[:, c, :], in_=w1_d[c])
            for c in range(8):
                nc.sync.dma_start(out=w2[:, c, :], in_=w2_d[c])
            bq = const.tile([P, 2], F32)
            b1 = const.tile([P, 8], F32)
            for c in range(2):
                nc.sync.dma_start(out=bq[:, c : c + 1], in_=bq_d[c])
            for c in range(8):
                nc.sync.dma_start(out=b1[:, c : c + 1], in_=b1_d[c])
            bkv = const.tile([1, 2 * C], BF16)
            bproj = const.tile([1, C], BF16)
            bfc2 = const.tile([1, C], BF16)
            if has_bkv:
                nc.sync.dma_start(out=bkv[:], in_=bkv_d[:])
            if has_bproj:
                nc.sync.dma_start(out=bproj[:], in_=bproj_d[:])
            if has_bfc2:
                nc.sync.dma_start(out=bfc2[:], in_=bfc2_d[:])

            # ---- residents ----
            x_sb = xres.tile([P, NTILES, C], F32)
            h1_sb = xres.tile([P, NTILES, C], F32)
            xhatT = xres.tile([P, 2, N], BF16)  # LN1(x)^T, feature-major
            qT = xres.tile([P, 2, N], BF16)  # q^T, feature-major

            # attention accumulators (live across phase A)
            attn_ps = [
                ps_attn.tile([P, C], F32, name=f"attn_ps{i}") for i in range(2)
            ]

            # =============== Phase A: LN1 + transpose + kv + attn accum =====
            def ln_normalize(src_ap, dst_bf16, mv, rs):
                # dst = (src - mean) * rstd   (cast to bf16)
                nc.vector.tensor_scalar(
                    out=dst_bf16,
                    in0=src_ap,
                    scalar1=mv[:, 0:1],
                    scalar2=rs,
                    op0=ALU.subtract,
                    op1=ALU.mult,
                )

            def ln_stats_batch(src_tiles, mv4, rs4):
                """src_tiles: list of (idx, AP[P, C]). Writes mv4[P, 4, 2], rs4[P, 4]."""
                for s, (_, src) in enumerate(src_tiles):
                    st = stats.tile([P, 6], F32, tag="bn")
                    nc.vector.bn_stats(out=st[:], in_=src)
                    nc.vector.bn_aggr(out=mv4[:, s, :], in_=st[:])
                # std4 = sqrt(var + eps); rs4 = 1/std4
                std4 = stats.tile([P, 4], F32, tag="std")
                nc.scalar.activation(
                    out=std4[:],
                    in_=mv4[:, :, 1],
                    func=AF.Sqrt,
                    bias=eps_t[:],
                )
                nc.vector.reciprocal(out=rs4[:], in_=std4[:])

            for g in range(NTILES // 4):  # groups of 4 token tiles
                idxs = [g * 4 + s for s in range(4)]
                for i in idxs:
                    nc.sync.dma_start(
                        out=x_sb[:, i, :], in_=x_d[i * P : (i + 1) * P, :]
                    )
                mv4 = stats.tile([P, 4, 2], F32, tag="mv")
                rs4 = stats.tile([P, 4], F32, tag="rs")
                ln_stats_batch([(i, x_sb[:, i, :]) for i in idxs], mv4, rs4)
                for s, i in enumerate(idxs):
                    xhat = work.tile([P, C], BF16, tag="xhat")
                    ln_normalize(x_sb[:, i, :], xhat[:], mv4[:, s, :], rs4[:, s : s + 1])
                    # transpose to feature-major: regular matmul against I
                    # (xhat.T @ I) -- same cycles as transpose-mode but stays
                    # compatible with walrus' LDW optimization.
                    for c in range(2):
                        tp = ps_tp.tile([P, P], F32, tag="tp")
                        nc.tensor.matmul(
                            tp[:],
                            xhat[:, c * P : (c + 1) * P],
                            ident[:],
                            start=True,
                            stop=True,
                        )
                        dst = xhatT[:, c, i * P : (i + 1) * P]
                        if c == 0:
                            nc.scalar.copy(out=dst, in_=tp[:])
                        else:
                            nc.vector.tensor_copy(out=dst, in_=tp[:])
                    # kv = xhat @ Wkv  (token-major out [P, 512])
                    kv_ps = ps_big.tile([P, 512], F32, tag="big")
                    nc.tensor.matmul(
                        kv_ps[:],
                        xhatT[:, 0, i * P : (i + 1) * P],
                        wqkv[:, 0, C : 3 * C],
                        start=True,
                        stop=False,
                    )
                    nc.tensor.matmul(
                        kv_ps[:],
                        xhatT[:, 1, i * P : (i + 1) * P],
                        wqkv[:, 1, C : 3 * C],
                        start=False,
                        stop=not has_bkv,
                    )
                    if has_bkv:
                        nc.tensor.matmul(
                            kv_ps[:], ones_row[:], bkv[:], start=False, stop=True
                        )
                    kv_sb = kvp.tile([P, 512], BF16, tag="kv")
                    nc.scalar.copy(out=kv_sb[:], in_=kv_ps[:])
                    # attn accumulation: attn[half] += k[:,half].T @ v
                    k_ap = kv_sb[:, 0:C]
                    v_ap = kv_sb[:, C : 2 * C]
                    for half in range(2):
                        nc.tensor.matmul(
                            attn_ps[half][:, :],
                            k_ap[:, half * P : (half + 1) * P],
                            v_ap,
                            start=(i == 0),
                            stop=(i == NTILES - 1),
                        )

            # =============== Phase A2: qT (feature-major q) ==================
            for n in range(NCHUNKS):
                sl = slice(n * 512, (n + 1) * 512)
                for fc in range(2):
                    q_ps = ps_big.tile([P, 512], F32, tag="big")
                    for kc in range(2):
                        nc.tensor.matmul(
                            q_ps[:],
                            wqkv[:, kc, fc * P : (fc + 1) * P],
                            xhatT[:, kc, sl],
                            start=(kc == 0),
                            stop=(kc == 1),
                        )
                    # evict with q bias (per-partition) + cast bf16
                    if fc == 0:
                        nc.scalar.activation(
                            out=qT[:, fc, sl],
                            in_=q_ps[:],
                            func=AF.Identity,
                            bias=bq[:, fc : fc + 1],
                        )
                    else:
                        nc.vector.tensor_scalar(
                            out=qT[:, fc, sl],
                            in0=q_ps[:],
                            scalar1=bq[:, fc : fc + 1],
                            scalar2=None,
                            op0=ALU.add,
                        )

            # =============== Phase B: softmax + block-diag attn^T ============
            Bd = const.tile([P, 2, P], BF16)
            nc.vector.memset(Bd[:], 0.0)
            BdT = const.tile([P, 2, P], BF16)
            nc.vector.memset(BdT[:], 0.0)
            for half in range(2):
                a_sb = work.tile([P, HD], F32, tag="attn")
                for h in range(4):
                    hh = half * 4 + h
                    nc.vector.tensor_copy(
                        out=a_sb[h * HD : (h + 1) * HD, :],
                        in_=attn_ps[half][
                            h * HD : (h + 1) * HD, hh * HD : (hh + 1) * HD
                        ],
                    )
                negmax = stats.tile([P, 1], F32, tag="negmax")
                nc.vector.tensor_reduce(
                    out=negmax[:], in_=a_sb[:], axis=AX.X, op=ALU.max, negate=True
                )
                exps = work.tile([P, HD], F32, tag="exps")
                nc.scalar.activation(
                    out=exps[:], in_=a_sb[:], func=AF.Exp, bias=negmax[:]
                )
                ssum = stats.tile([P, 1], F32, tag="ssum")
                nc.vector.tensor_reduce(
                    out=ssum[:], in_=exps[:], axis=AX.X, op=ALU.add
                )
                rec = stats.tile([P, 1], F32, tag="rec")
                nc.vector.reciprocal(out=rec[:], in_=ssum[:])
                attn_n = work.tile([P, HD], F32, tag="attn_n")
                nc.vector.tensor_scalar(
                    out=attn_n[:],
                    in0=exps[:],
                    scalar1=rec[:],
                    scalar2=None,
                    op0=ALU.mult,
                )
                attn_t = work.tile([P, HD], F32, tag="attn_t")
                nc.vector.transpose(out=attn_t[:], in_=attn_n[:])
                for h in range(4):
                    nc.vector.tensor_copy(
                        out=Bd[h * HD : (h + 1) * HD, half, h * HD : (h + 1) * HD],
                        in_=attn_t[h * HD : (h + 1) * HD, :],
                    )
                    nc.gpsimd.tensor_copy(
                        out=BdT[h * HD : (h + 1) * HD, half, h * HD : (h + 1) * HD],
                        in_=attn_n[h * HD : (h + 1) * HD, :],
                    )

            # E[half] = blockdiag(attn)[half] @ Wproj[half] -> proj fused into
            # the second attention einsum (y and its eviction disappear).
            E_sb = const.tile([P, 2, C], BF16)
            for half in range(2):
                e_ps = ps_small.tile([P, C], F32, tag="small")
                nc.tensor.matmul(
                    e_ps[:], BdT[:, half, :], wproj[:, half, :], start=True, stop=True
                )
                nc.vector.tensor_copy(out=E_sb[:, half, :], in_=e_ps[:])

            # =============== Phase C1: y, proj, h1, LN2 stats (all chunks) ===
            # Keeping ALL LN2 sqrt work in one batched ACT op (and all gelus
            # after it) avoids the Sqrt<->Gelu table-set thrash (~2.7us per
            # switch) that showed up as 18 ACT_TABLE_LOADs in the profile.
            mv32 = xres.tile([P, NTILES, 2], F32)
            rs32 = xres.tile([P, NTILES], F32)
            def c1_chunk(n):
                for s in range(4):
                    i = n * 4 + s
                    p_ps = ps_small.tile([P, C], F32, tag="small", name=f"pp{i}")
                    nc.tensor.matmul(
                        p_ps[:],
                        qT[:, 0, i * P : (i + 1) * P],
                        E_sb[:, 0, :],
                        start=True,
                        stop=False,
                    )
                    nc.tensor.matmul(
                        p_ps[:],
                        qT[:, 1, i * P : (i + 1) * P],
                        E_sb[:, 1, :],
                        start=False,
                        stop=not has_bproj,
                    )
                    if has_bproj:
                        nc.tensor.matmul(
                            p_ps[:], ones_row[:], bproj[:], start=False, stop=True
                        )
                    # h1 = x + attn_out
                    nc.vector.tensor_tensor(
                        out=h1_sb[:, i, :], in0=p_ps[:], in1=x_sb[:, i, :], op=ALU.add
                    )
                    st = stats.tile([P, 6], F32, tag="bn", name=f"st{i}")
                    nc.vector.bn_stats(out=st[:], in_=h1_sb[:, i, :])
                    nc.vector.bn_aggr(out=mv32[:, i, :], in_=st[:])

            def half_rstd(g):
                # batched rstd for subtiles g*16 .. g*16+15
                sl = slice(g * 16, (g + 1) * 16)
                std16 = stats.tile([P, 16], F32, tag="std16", name=f"sd{g}")
                nc.scalar.activation(
                    out=std16[:], in_=mv32[:, sl, 1], func=AF.Sqrt, bias=eps_t[:]
                )
                nc.vector.reciprocal(out=rs32[:, sl], in_=std16[:])

            # Emit C1 in two halves with the rstd for each half immediately
            # after: C2 of half 0 can then overlap C1 of half 1, keeping the
            # PE warm through the DVE-heavy C1 stretch.
            c1_chunk(0), c1_chunk(1), c1_chunk(2), c1_chunk(3)
            half_rstd(0)
            c1_chunk(4), c1_chunk(5), c1_chunk(6), c1_chunk(7)
            half_rstd(1)

            # =============== Phase C2: LN2 apply, MLP, residuals ============
            for n in range(NCHUNKS):
                xhat2T = big.tile([P, 2, 512], mlp_dt, tag="x2T")
                for s in range(4):
                    i = n * 4 + s
                    xhat2 = work.tile([P, C], BF16, tag="xhat2")
                    ln_normalize(
                        h1_sb[:, i, :], xhat2[:], mv32[:, i, :], rs32[:, i : i + 1]
                    )
                    for c in range(2):
                        tp = ps_tp.tile([P, P], F32, tag="tp")
                        nc.tensor.matmul(
                            tp[:],
                            xhat2[:, c * P : (c + 1) * P],
                            ident[:],
                            start=True,
                            stop=True,
                        )
                        dst = xhat2T[:, c, s * P : (s + 1) * P]
                        if c == 0:
                            nc.scalar.copy(out=dst, in_=tp[:])
                        else:
                            nc.vector.tensor_copy(out=dst, in_=tp[:])

                # fc1 + gelu (feature-major hidden)
                g1T = big.tile([P, 8, 512], mlp_dt, tag="g1T")
                for hc in range(8):
                    f_ps = ps_big.tile([P, 512], F32, tag="big")
                    if USE_FP8:
                        nc.tensor.matmul(
                            f_ps[:],
                            w1[:, :, hc * P : (hc + 1) * P],
                            xhat2T[:, :, :],
                            start=True,
                            stop=True,
                            perf_mode=mybir.MatmulPerfMode.DoubleRow,
                        )
                    else:
                        for kc in range(2):
                            nc.tensor.matmul(
                                f_ps[:],
                                w1[:, kc, hc * P : (hc + 1) * P],
                                xhat2T[:, kc, :],
                                start=(kc == 0),
                                stop=(kc == 1),
                            )
                    nc.scalar.activation(
                        out=g1T[:, hc, :],
                        in_=f_ps[:],
                        func=AF.Gelu,
                        bias=b1[:, hc : hc + 1],
                        scale=(1.0 / FP8_SCALE) if USE_FP8 else 1.0,
                    )

                # fc2 + final residuals, per subtile
                for s in range(4):
                    i = n * 4 + s
                    m_ps = ps_small.tile([P, C], F32, tag="small")
                    if USE_FP8:
                        for j in range(4):
                            nc.tensor.matmul(
                                m_ps[:],
                                g1T[:, 2 * j : 2 * j + 2, s * P : (s + 1) * P],
                                w2[:, 2 * j : 2 * j + 2, :],
                                start=(j == 0),
                                stop=(j == 3 and not has_bfc2),
                                perf_mode=mybir.MatmulPerfMode.DoubleRow,
                            )
                    else:
                        for hc in range(8):
                            nc.tensor.matmul(
                                m_ps[:],
                                g1T[:, hc, s * P : (s + 1) * P],
                                w2[:, hc, :],
                                start=(hc == 0),
                                stop=(hc == 7 and not has_bfc2),
                            )
                    if has_bfc2:
                        nc.tensor.matmul(
                            m_ps[:], ones_row[:], bfc2[:], start=False, stop=True
                        )
                    t1 = outp.tile([P, C], F32, tag="t1")
                    if USE_FP8:
                        nc.vector.scalar_tensor_tensor(
                            out=t1[:],
                            in0=m_ps[:],
                            scalar=1.0 / FP8_SCALE,
                            in1=h1_sb[:, i, :],
                            op0=ALU.mult,
                            op1=ALU.add,
                        )
                    else:
                        nc.vector.tensor_tensor(
                            out=t1[:], in0=m_ps[:], in1=h1_sb[:, i, :], op=ALU.add
                        )
                    o_t = outp.tile([P, C], F32, tag="ot")
                    nc.gpsimd.tensor_tensor(
                        out=o_t[:], in0=t1[:], in1=x_sb[:, i, :], op=ALU.add
                    )
                    nc.sync.dma_start(out=out_d[i * P : (i + 1) * P, :], in_=o_t[:])

    _split_sync_waits(nc)
    return nc


_CACHE = {}


def _get_nc(key):
    if key not in _CACHE:
        _CACHE[key] = _build_nc(*key)
    return _CACHE[key]


def _prep_inputs(inputs):
    """Host-side weight folding.  Returns (shared_in_map, has_flags)."""
    f32 = lambda k: np.asarray(inputs[k], dtype=np.float32)
    qkv_w, qkv_b = f32("qkv_w"), f32("qkv_b")
    proj_w, proj_b = f32("proj_w"), f32("proj_b")
    ln1_g, ln1_b = f32("ln1_g"), f32("ln1_b")
    ln2_g, ln2_b = f32("ln2_g"), f32("ln2_b")
    fc1_w, fc1_b = f32("fc1_w"), f32("fc1_b")
    fc2_w, fc2_b = f32("fc2_w"), f32("fc2_b")

    scale = HD ** (-0.5)

    # Fold LN1 affine into qkv: LN1(x)@W+b = xhat@(g*W) + (ln1_b@W + b)
    wqkv_f = ln1_g[:, None] * qkv_w
    bqkv_f = ln1_b @ qkv_w + qkv_b
    # Fold channel-attention scale into k
    wqkv_f[:, C : 2 * C] *= scale
    bqkv_f[C : 2 * C] *= scale
    # Fold LN2 affine into fc1
    w1_f = ln2_g[:, None] * fc1_w
    b1_f = ln2_b @ fc1_w + fc1_b

    bq = bqkv_f[0:C]
    bkv = bqkv_f[C : 3 * C]

    has_flags = (
        bool(np.any(bkv != 0)),
        bool(np.any(proj_b != 0)),
        bool(np.any(fc2_b != 0)),
    )

    shared = {
        "wqkv": wqkv_f.reshape(2, P, 3 * C).astype(NP_BF16),
        "wproj": proj_w.reshape(2, P, C).astype(NP_BF16),
        "w1": (w1_f * FP8_SCALE).reshape(2, P, HID).astype(NP_FP8)
        if USE_FP8
        else w1_f.reshape(2, P, HID).astype(NP_BF16),
        "w2": (fc2_w * FP8_SCALE).reshape(8, P, C).astype(NP_FP8)
        if USE_FP8
        else fc2_w.reshape(8, P, C).astype(NP_BF16),
        "bq": bq.reshape(2, P, 1).astype(np.float32),
        "b1": b1_f.reshape(8, P, 1).astype(np.float32),
        "bkv": bkv.reshape(1, 2 * C).astype(NP_BF16),
        "bproj": proj_b.reshape(1, C).astype(NP_BF16),
        "bfc2": (fc2_b * (FP8_SCALE if USE_FP8 else 1.0)).reshape(1, C).astype(NP_BF16),
    }
    return shared, has_flags


def kernel(x, **weights):
    x = np.asarray(x, dtype=np.float32)
    shared, has_flags = _prep_inputs(weights)
    nc = _get_nc(has_flags)
    in_maps = [dict(shared, x=np.ascontiguousarray(x[b])) for b in range(B)]
    res = run_bass_kernel_spmd(nc, in_maps, list(range(B)))
    out = np.stack([res.results[b]["out"] for b in range(B)], axis=0)
    return out.astype(np.float32)
